# revision 30
# baseline (speedup 1.0000x reference)
"""Trainium2 Bass kernel for Detr3D cross-attention.

Sharding: query-parallel across 8 NeuronCores (128 queries per core).
Feature pyramids are replicated per core in a channel-last flat layout
(rows of 256 contiguous floats per spatial position), so the sparse
sampling stage is a per-camera indirect DMA gather (dma_gather) with
indices computed on-device from reference_points @ lidar2img.

Per-core device program:
  1. rpc = rp_h @ M^T via one PE matmul (queries on partitions).
  2. DVE chain computes sample coords and flat gather indices first
     (x and y fused into 48-wide tiles), folds them into dma_gather's
     wrapped int16 index layout using constant 0/1 "fold" matmuls on
     the PE, and launches the 6 per-camera gathers as early as
     possible (1024 rows x 2KB each; one row = a (query, level,
     y-tap); 512 floats cover the x0 and x0+1 taps at once).
  3. While the gathers stream, DVE computes bilinear weights, masks and
     sigmoid(attn) scaling, and the PE computes the positional-encoder
     branch.
  4. Per camera: DVE scales gathered rows by the combined weights and
     reduces the 16 (level,ytap,xtap) slots per query with a pairwise
     add tree, accumulating across cameras.
  5. Tail: W_out projection, residual adds, W_fin projection and final
     LayerNorm; each core emits its own (128, 64) output slice.

The host reassembles the 8 slices into the full (1024, 1, 64) output.
"""

import numpy as np

# ---------------------------------------------------------------- constants
Q, B, N, C = 1024, 1, 6, 256
NCORES = 8
QPC = Q // NCORES                       # 128 queries per core
LVL = [(116, 200), (58, 100), (29, 50), (15, 25)]
LV_BASE = [0, 23200, 29000, 30450]
CAM_ROWS = 30825                        # rows per camera (sum H*W)
FEAT_ROWS = N * CAM_ROWS + 135          # pad so 2KB reads never run off the end
IMG_H, IMG_W = 928.0, 1600.0
EPS = 1e-5
NPAIR = 24                              # (cam, level) pairs
MAGIC = 8388608.0                       # 2^23: round-to-nearest trick

_CACHE = {}


# ---------------------------------------------------------------- host prep
def _host_shared(inputs):
    """Inputs identical on every core."""
    feats = [inputs[f"feat{i}"] for i in range(4)]
    featT = np.zeros((FEAT_ROWS, C), np.float32)
    for c in range(N):
        for l, (H, W) in enumerate(LVL):
            r0 = c * CAM_ROWS + LV_BASE[l]
            featT[r0:r0 + H * W] = feats[l][0, c].reshape(C, H * W).T
    l2i = np.asarray(inputs["lidar2img"], np.float32)
    # mats[k, coord*6+cam] = l2i[0, cam, coord, k]   (coords x,y,z)
    mats = np.ascontiguousarray(
        np.transpose(l2i[0][:, 0:3, :], (2, 1, 0)).reshape(4, 18))

    def c24(fn):
        row = np.array([fn(lv) for cc in range(N) for lv in range(4)], np.float32)
        return np.ascontiguousarray(np.broadcast_to(row, (128, NPAIR)))

    def c48(fx, fy):
        row = np.array([f(lv) for f in (fx, fy) for cc in range(N) for lv in range(4)],
                       np.float32)
        return np.ascontiguousarray(np.broadcast_to(row, (128, 2 * NPAIR)))

    sxy_r = c48(lambda l: LVL[l][1] / IMG_W, lambda l: LVL[l][0] / IMG_H)
    wh_r = c48(lambda l: float(LVL[l][1]), lambda l: float(LVL[l][0]))
    whm1_r = c48(lambda l: float(LVL[l][1] - 1), lambda l: float(LVL[l][0] - 1))
    wt_r = c24(lambda l: float(LVL[l][1]))
    base_r = c24(lambda l: float(LV_BASE[l]))

    sfold = np.zeros((128, 1024), np.float32)
    for j in range(8):
        for p in range(16):
            sfold[16 * j + p, 128 * j + 16 * np.arange(8) + p] = 1.0
    i128 = np.eye(128, dtype=np.float32)
    i16x = np.ascontiguousarray(np.tile(i128, (1, 16)))   # (128, 2048)

    def repl(v, w):
        v = np.asarray(v, np.float32).reshape(1, w)
        return np.ascontiguousarray(np.broadcast_to(v, (128, w)))

    shared = dict(
        featT=featT, mats=mats,
        sxy_r=sxy_r, wh_r=wh_r, whm1_r=whm1_r, wt_r=wt_r, base_r=base_r,
        sfold=sfold, i128=i128, i16x=i16x,
        wqe=np.asarray(inputs["W_qe"], np.float32),
        wattn=np.asarray(inputs["W_attn"], np.float32),
        wout=np.asarray(inputs["W_out"], np.float32),
        pw1=np.asarray(inputs["pe_w1"], np.float32),
        pw2=np.asarray(inputs["pe_w2"], np.float32),
        wfin=np.asarray(inputs["W_fin"], np.float32),
        bqe_r=repl(inputs["b_qe"], 256),
        battn_r=repl(inputs["b_attn"], 24),
        bout_r=repl(inputs["b_out"], 256),
        pb1_r=repl(inputs["pe_b1"], 256),
        pg1_r=repl(inputs["pe_g1"], 256),
        pbe1_r=repl(inputs["pe_be1"], 256),
        pb2_r=repl(inputs["pe_b2"], 256),
        pg2_r=repl(inputs["pe_g2"], 256),
        pbe2_r=repl(inputs["pe_be2"], 256),
        bfin_r=repl(inputs["b_fin"], 64),
        gn_r=repl(inputs["g_norm"], 64),
        bn_r=repl(inputs["b_norm"], 64),
    )
    return shared


def _host_per_core(inputs, ci):
    qs, qe = ci * QPC, (ci + 1) * QPC
    qT = np.ascontiguousarray(np.asarray(inputs["query"], np.float32)[qs:qe, 0, :].T)
    qpT = np.ascontiguousarray(np.asarray(inputs["query_pos"], np.float32)[qs:qe, 0, :].T)
    rp = np.asarray(inputs["reference_points"], np.float32)[0, qs:qe, :]   # (128,3)
    rp_hT = np.concatenate([rp.T, np.ones((1, QPC), np.float32)], axis=0)  # (4,128)
    return dict(qT=qT, qpT=qpT, rp_hT=np.ascontiguousarray(rp_hT))


def make_in_maps(inputs):
    shared = _host_shared(inputs)
    return [dict(shared, **_host_per_core(inputs, ci)) for ci in range(NCORES)]


# ---------------------------------------------------------------- device
def _sub(t, off, dims):
    """Custom sub-AP of a pool tile: same partition dim, new free dims."""
    import concourse.bass as bass
    return bass.AP(t.tensor, t.offset + off, [list(t.ap[0])] + [list(d) for d in dims])


def build_nc():
    import concourse.bass as bass
    import concourse.bacc as bacc
    import concourse.mybir as mybir
    import concourse.tile as tile

    f32 = mybir.dt.float32
    i16 = mybir.dt.int16
    Alu = mybir.AluOpType
    Act = mybir.ActivationFunctionType

    nc = bacc.Bacc("TRN2", target_bir_lowering=False, debug=False,
                   enable_asserts=False, num_devices=NCORES)

    def din(name, shape):
        return nc.dram_tensor(name, list(shape), f32, kind="ExternalInput").ap()

    featT = din("featT", (FEAT_ROWS, C))
    mats = din("mats", (4, 18))
    rph_d = din("rp_hT", (4, 128))
    sxy_d, wh_d, whm1_d = din("sxy_r", (128, 48)), din("wh_r", (128, 48)), din("whm1_r", (128, 48))
    wt_d, base_d = din("wt_r", (128, 24)), din("base_r", (128, 24))
    sfold_d = din("sfold", (128, 1024))
    i128_d = din("i128", (128, 128))
    i16x_d = din("i16x", (128, 2048))
    qT_d, qpT_d = din("qT", (64, 128)), din("qpT", (64, 128))
    wqe_d, wattn_d = din("wqe", (64, 256)), din("wattn", (256, 24))
    wout_d, pw1_d = din("wout", (256, 256)), din("pw1", (3, 256))
    pw2_d, wfin_d = din("pw2", (256, 256)), din("wfin", (256, 64))
    bqe_d, battn_d = din("bqe_r", (128, 256)), din("battn_r", (128, 24))
    bout_d = din("bout_r", (128, 256))
    pb1_d, pg1_d, pbe1_d = din("pb1_r", (128, 256)), din("pg1_r", (128, 256)), din("pbe1_r", (128, 256))
    pb2_d, pg2_d, pbe2_d = din("pb2_r", (128, 256)), din("pg2_r", (128, 256)), din("pbe2_r", (128, 256))
    bfin_d, gn_d, bn_d = din("bfin_r", (128, 64)), din("gn_r", (128, 64)), din("bn_r", (128, 64))

    out_d = nc.dram_tensor("out", [QPC, 64], f32, kind="ExternalOutput").ap()

    from contextlib import ExitStack
    with tile.TileContext(nc) as tc, ExitStack() as stack:
        cp = stack.enter_context(tc.tile_pool(name="consts", bufs=1))
        wp = stack.enter_context(tc.tile_pool(name="work", bufs=1))
        gp = stack.enter_context(tc.tile_pool(name="gbuf", bufs=2))
        pp = stack.enter_context(tc.tile_pool(name="psum", bufs=4, space="PSUM"))

        def load(dram_ap, shape, name):
            t = cp.tile(shape, f32, name=name)
            nc.sync.dma_start(out=t[:, :], in_=dram_ap)
            return t

        def load2(dram_ap, shape, name):
            # tail-only constants go on the second HWDGE ring (ACT engine)
            t = cp.tile(shape, f32, name=name)
            nc.scalar.dma_start(out=t[:, :], in_=dram_ap)
            return t

        # chain-critical consts first
        mats_s = load(mats, (4, 18), "mats_s")
        rph_s = load(rph_d, (4, 128), "rph_s")
        sxy_s = load(sxy_d, (128, 48), "sxy_s")
        wh_s = load(wh_d, (128, 48), "wh_s")
        whm1_s = load(whm1_d, (128, 48), "whm1_s")
        wt_s = load(wt_d, (128, 24), "wt_s")
        base_s = load(base_d, (128, 24), "base_s")
        sfold_s = load(sfold_d, (128, 1024), "sfold_s")
        i128_s = load(i128_d, (128, 128), "i128_s")
        i16x_s = load2(i16x_d, (128, 2048), "i16x_s")
        qT_s = load(qT_d, (64, 128), "qT_s")
        qpT_s = load(qpT_d, (64, 128), "qpT_s")
        wqe_s = load(wqe_d, (64, 256), "wqe_s")
        wattn0 = load(wattn_d[0:128, :], (128, 24), "wattn0")
        wattn1 = load(wattn_d[128:256, :], (128, 24), "wattn1")
        wout0 = load2(wout_d[0:128, :], (128, 256), "wout0")
        wout1 = load2(wout_d[128:256, :], (128, 256), "wout1")
        pw1_s = load2(pw1_d, (3, 256), "pw1_s")
        pw2_0 = load2(pw2_d[0:128, :], (128, 256), "pw2_0")
        pw2_1 = load2(pw2_d[128:256, :], (128, 256), "pw2_1")
        wfin0 = load2(wfin_d[0:128, :], (128, 64), "wfin0")
        wfin1 = load2(wfin_d[128:256, :], (128, 64), "wfin1")
        bqe_s = load(bqe_d, (128, 256), "bqe_s")
        battn_s = load(battn_d, (128, 24), "battn_s")
        bout_s = load2(bout_d, (128, 256), "bout_s")
        pb1_s, pg1_s, pbe1_s = load2(pb1_d, (128, 256), "pb1_s"), load2(pg1_d, (128, 256), "pg1_s"), load(pbe1_d, (128, 256), "pbe1_s")
        pb2_s, pg2_s, pbe2_s = load2(pb2_d, (128, 256), "pb2_s"), load(pg2_d, (128, 256), "pg2_s"), load(pbe2_d, (128, 256), "pbe2_s")
        bfin_s, gn_s, bn_s = load2(bfin_d, (128, 64), "bfin_s"), load(gn_d, (128, 64), "gn_s"), load(bn_d, (128, 64), "bn_s")

        V = nc.vector
        S = nc.scalar
        T = nc.tensor
        GS = nc.gpsimd

        def vt(shape, name, dtype=f32, pool=wp, **kw):
            return pool.tile(list(shape), dtype, name=name, **kw)

        # ---------------- A: projection -----------------------------------
        rpc_p = pp.tile([128, 18], f32, name="rpc_p", tag="ps")
        T.matmul(rpc_p[:, :], lhsT=rph_s[:, :], rhs=mats_s[:, :], start=True, stop=True)
        RPC = vt((128, 18), "RPC")
        V.tensor_copy(out=RPC[:, :], in_=rpc_p[:, :])
        Xc, Yc, Zc = RPC[:, 0:6], RPC[:, 6:12], RPC[:, 12:18]

        zc = vt((128, 6), "zc")
        V.tensor_scalar_max(out=zc[:, :], in0=Zc, scalar1=EPS)
        rz = vt((128, 6), "rz")
        V.reciprocal(out=rz[:, :], in_=zc[:, :])
        XYq = vt((128, 12), "XYq")          # [x_img(6) | y_img(6)]
        V.tensor_tensor(out=XYq[:, 0:6], in0=Xc, in1=rz[:, :], op=Alu.mult)
        V.tensor_tensor(out=XYq[:, 6:12], in0=Yc, in1=rz[:, :], op=Alu.mult)

        # ---------------- B: index path (48-wide: [x(24) | y(24)]) --------
        def bc12(t):   # (128,12) -> (128,12,4) broadcast over levels
            return _sub(t, 0, [[1, 12], [0, 4]])

        def w3(t):     # (128,48) viewed as (128,12,4)
            return _sub(t, 0, [[4, 12], [1, 4]])

        xy = vt((128, 48), "xy")
        V.scalar_tensor_tensor(out=w3(xy), in0=bc12(XYq), scalar=1.0, in1=w3(sxy_s),
                               op0=Alu.mult, op1=Alu.mult)
        V.tensor_scalar_add(out=xy[:, :], in0=xy[:, :], scalar1=-0.5)

        t48 = vt((128, 48), "t48")
        V.tensor_scalar_add(out=t48[:, :], in0=xy[:, :], scalar1=1.0)
        fl48 = vt((128, 48), "fl48")
        V.tensor_scalar(out=fl48[:, :], in0=t48[:, :], scalar1=MAGIC, scalar2=MAGIC,
                        op0=Alu.add, op1=Alu.subtract)
        cr48 = vt((128, 48), "cr48")
        V.tensor_tensor(out=cr48[:, :], in0=fl48[:, :], in1=t48[:, :], op=Alu.is_gt)
        V.tensor_tensor(out=fl48[:, :], in0=fl48[:, :], in1=cr48[:, :], op=Alu.subtract)
        # fl48 = floor(xy + 1) = floor(xy) + 1
        ii0 = vt((128, 48), "ii0")          # [ix | iy0] clipped
        V.tensor_scalar(out=ii0[:, :], in0=fl48[:, :], scalar1=-1.0, scalar2=0.0,
                        op0=Alu.add, op1=Alu.max)
        V.tensor_tensor(out=ii0[:, :], in0=ii0[:, :], in1=whm1_s[:, :], op=Alu.min)
        iy1 = vt((128, 24), "iy1")          # clip(y0+1) = clip(fl48_y)
        V.tensor_scalar_max(out=iy1[:, :], in0=fl48[:, 24:48], scalar1=0.0)
        V.tensor_tensor(out=iy1[:, :], in0=iy1[:, :], in1=whm1_s[:, 24:48], op=Alu.min)

        ix, iy0 = ii0[:, 0:24], ii0[:, 24:48]
        fold_src = vt((128, 48), "fold_src")
        for yt, iy in ((0, iy0), (1, iy1[:, :])):
            dst = fold_src[:, 24 * yt:24 * yt + 24]
            V.tensor_tensor(out=dst, in0=iy, in1=wt_s[:, :], op=Alu.mult)
            V.tensor_tensor(out=dst, in0=dst, in1=ix, op=Alu.add)
            V.tensor_tensor(out=dst, in0=dst, in1=base_s[:, :], op=Alu.add)

        idx_p = pp.tile([128, 384], f32, name="idx_p", tag="psidx", bufs=1)
        for j in range(8):
            T.matmul(idx_p[:, 48 * j:48 * j + 48],
                     lhsT=sfold_s[:, 128 * j:128 * j + 128],
                     rhs=fold_src[:, :], start=True, stop=True)

        mega = vt((128, 384), "mega", dtype=i16)
        for yt in range(2):
            # dest col = 64c + 16lv + 8yt + j ; src col = 48j + 24yt + 4c + lv
            V.tensor_copy(
                out=_sub(mega, 8 * yt, [[64, 6], [16, 4], [1, 8]]),
                in_=_sub(idx_p, 24 * yt, [[4, 6], [1, 4], [48, 8]]))

        # ---------------- gathers (launch ASAP) ---------------------------
        g_tiles = []
        for cam in range(N):
            g_t = gp.tile([128, 4096], mybir.dt.float32r, name=f"g{cam}", tag="G", bufs=3)
            in_ap = bass.AP(featT.tensor, cam * CAM_ROWS * C,
                            [[C, CAM_ROWS + 130], [1, 512]]).bitcast(mybir.dt.float32r)
            GS.dma_gather(
                out_ap=_sub(g_t, 0, [[512, 8], [1, 512]]),
                in_ap=in_ap,
                idxs_ap=mega[:, 64 * cam:64 * cam + 64],
                num_idxs=1024, num_idxs_reg=1024,
                elem_size=512, elem_step=C)
            g_tiles.append(g_t)

        # ---------------- C: weights (overlap with gathers) ---------------
        v0 = vt((128, 48), "v0")
        tmp48 = vt((128, 48), "tmp48")
        V.tensor_scalar(out=v0[:, :], in0=xy[:, :], scalar1=0.0, scalar2=None, op0=Alu.is_ge)
        V.tensor_tensor(out=tmp48[:, :], in0=xy[:, :], in1=wh_s[:, :], op=Alu.is_lt)
        V.tensor_tensor(out=v0[:, :], in0=v0[:, :], in1=tmp48[:, :], op=Alu.mult)
        v1 = vt((128, 48), "v1")
        V.tensor_scalar(out=v1[:, :], in0=xy[:, :], scalar1=-1.0, scalar2=None, op0=Alu.is_ge)
        V.tensor_tensor(out=tmp48[:, :], in0=xy[:, :], in1=whm1_s[:, :], op=Alu.is_lt)
        V.tensor_tensor(out=v1[:, :], in0=v1[:, :], in1=tmp48[:, :], op=Alu.mult)
        sh = vt((128, 24), "sh")
        V.tensor_scalar(out=sh[:, :], in0=xy[:, 0:24], scalar1=0.0, scalar2=None, op0=Alu.is_lt)
        fr48 = vt((128, 48), "fr48")
        V.tensor_tensor(out=fr48[:, :], in0=t48[:, :], in1=fl48[:, :], op=Alu.subtract)
        w048 = vt((128, 48), "w048")
        V.tensor_scalar(out=w048[:, :], in0=fr48[:, :], scalar1=-1.0, scalar2=1.0,
                        op0=Alu.mult, op1=Alu.add)

        # mask per cam: front & inbounds (strict)
        front = vt((128, 6), "front")
        V.tensor_scalar(out=front[:, :], in0=Zc, scalar1=EPS, scalar2=None, op0=Alu.is_gt)
        m1 = vt((128, 12), "m1")
        m2 = vt((128, 12), "m2")
        V.tensor_scalar(out=m1[:, :], in0=XYq[:, :], scalar1=0.0, scalar2=None, op0=Alu.is_gt)
        V.tensor_scalar(out=m2[:, 0:6], in0=XYq[:, 0:6], scalar1=IMG_W, scalar2=None, op0=Alu.is_lt)
        V.tensor_scalar(out=m2[:, 6:12], in0=XYq[:, 6:12], scalar1=IMG_H, scalar2=None, op0=Alu.is_lt)
        V.tensor_tensor(out=m1[:, :], in0=m1[:, :], in1=m2[:, :], op=Alu.mult)
        mask = vt((128, 6), "mask")
        V.tensor_tensor(out=mask[:, :], in0=m1[:, 0:6], in1=m1[:, 6:12], op=Alu.mult)
        V.tensor_tensor(out=mask[:, :], in0=mask[:, :], in1=front[:, :], op=Alu.mult)

        # qe / attention
        qsT = vt((64, 128), "qsT")
        V.tensor_tensor(out=qsT[:, :], in0=qT_s[:, :], in1=qpT_s[:, :], op=Alu.add)
        qe_p = pp.tile([128, 256], f32, name="qe_p", tag="ps")
        T.matmul(qe_p[:, :], lhsT=qsT[:, :], rhs=wqe_s[:, :], start=True, stop=True)
        qe = vt((128, 256), "qe")
        V.scalar_tensor_tensor(out=qe[:, :], in0=qe_p[:, :], scalar=0.0, in1=bqe_s[:, :],
                               op0=Alu.add, op1=Alu.add)
        qeT0_p = pp.tile([128, 128], f32, name="qeT0_p", tag="ps")
        T.transpose(qeT0_p[:, :], qe[:, 0:128], i128_s[:, :])
        qeT1_p = pp.tile([128, 128], f32, name="qeT1_p", tag="ps")
        T.transpose(qeT1_p[:, :], qe[:, 128:256], i128_s[:, :])
        qeT0 = vt((128, 128), "qeT0")
        V.tensor_copy(out=qeT0[:, :], in_=qeT0_p[:, :])
        qeT1 = vt((128, 128), "qeT1")
        V.tensor_copy(out=qeT1[:, :], in_=qeT1_p[:, :])
        attw_p = pp.tile([128, 24], f32, name="attw_p", tag="ps")
        T.matmul(attw_p[:, :], lhsT=qeT0[:, :], rhs=wattn0[:, :], start=True, stop=False)
        T.matmul(attw_p[:, :], lhsT=qeT1[:, :], rhs=wattn1[:, :], start=False, stop=True)
        attwb = vt((128, 24), "attwb")
        V.scalar_tensor_tensor(out=attwb[:, :], in0=attw_p[:, :], scalar=0.0,
                               in1=battn_s[:, :], op0=Alu.add, op1=Alu.add)
        sgm = vt((128, 24), "sgm")
        S.activation(out=sgm[:, :], in_=attwb[:, :], func=Act.Sigmoid)
        s_eff = vt((128, 24), "s_eff")
        V.scalar_tensor_tensor(out=_sub(s_eff, 0, [[4, 6], [1, 4]]),
                               in0=_sub(mask, 0, [[1, 6], [0, 4]]), scalar=1.0,
                               in1=_sub(sgm, 0, [[4, 6], [1, 4]]),
                               op0=Alu.mult, op1=Alu.mult)

        # final per-slot weights: w_all col = 16c + 4lv + 2yt + half
        wlo = vt((128, 24), "wlo")
        whi = vt((128, 24), "whi")
        tb = vt((128, 24), "tb")
        V.tensor_tensor(out=wlo[:, :], in0=w048[:, 0:24], in1=v0[:, 0:24], op=Alu.mult)
        V.tensor_tensor(out=tb[:, :], in0=fr48[:, 0:24], in1=v1[:, 0:24], op=Alu.mult)
        V.tensor_tensor(out=whi[:, :], in0=tb[:, :], in1=sh[:, :], op=Alu.mult)
        V.tensor_tensor(out=wlo[:, :], in0=wlo[:, :], in1=whi[:, :], op=Alu.add)
        V.tensor_tensor(out=whi[:, :], in0=tb[:, :], in1=whi[:, :], op=Alu.subtract)
        wy0v = vt((128, 24), "wy0v")
        V.tensor_tensor(out=wy0v[:, :], in0=w048[:, 24:48], in1=v0[:, 24:48], op=Alu.mult)
        wy1v = vt((128, 24), "wy1v")
        V.tensor_tensor(out=wy1v[:, :], in0=fr48[:, 24:48], in1=v1[:, 24:48], op=Alu.mult)
        sy0 = vt((128, 24), "sy0")
        V.tensor_tensor(out=sy0[:, :], in0=s_eff[:, :], in1=wy0v[:, :], op=Alu.mult)
        sy1 = vt((128, 24), "sy1")
        V.tensor_tensor(out=sy1[:, :], in0=s_eff[:, :], in1=wy1v[:, :], op=Alu.mult)
        w_all = vt((128, 96), "w_all")
        for (syt, yt) in ((sy0, 0), (sy1, 1)):
            for (wx, half) in ((wlo, 0), (whi, 1)):
                V.tensor_tensor(
                    out=_sub(w_all, 2 * yt + half, [[16, 6], [4, 4]]),
                    in0=_sub(syt, 0, [[4, 6], [1, 4]]),
                    in1=_sub(wx, 0, [[4, 6], [1, 4]]), op=Alu.mult)

        # ---------------- helpers ----------------------------------------
        def transpose2(src, name):
            t0p = pp.tile([128, 128], f32, name=f"{name}0p", tag="ps")
            T.transpose(t0p[:, :], src[:, 0:128], i128_s[:, :])
            t1p = pp.tile([128, 128], f32, name=f"{name}1p", tag="ps")
            T.transpose(t1p[:, :], src[:, 128:256], i128_s[:, :])
            t0 = vt((128, 128), f"{name}0")
            V.tensor_copy(out=t0[:, :], in_=t0p[:, :])
            t1 = vt((128, 128), f"{name}1")
            V.tensor_copy(out=t1[:, :], in_=t1p[:, :])
            return t0, t1

        def layer_norm(x, g_s, b_s, dim, name):
            mu = vt((128, 1), f"{name}_mu")
            V.tensor_reduce(out=mu[:, :], in_=x[:, :], axis=mybir.AxisListType.X, op=Alu.add)
            V.tensor_scalar_mul(out=mu[:, :], in0=mu[:, :], scalar1=1.0 / dim)
            xm = vt((128, dim), f"{name}_xm")
            V.tensor_scalar(out=xm[:, :], in0=x[:, :], scalar1=mu[:, :], scalar2=None,
                            op0=Alu.subtract)
            sq = vt((128, dim), f"{name}_sq")
            vs = vt((128, 1), f"{name}_vs")
            V.scalar_tensor_tensor(out=sq[:, :], in0=xm[:, :], scalar=0.0, in1=xm[:, :],
                                   op0=Alu.add, op1=Alu.mult, accum_out=vs[:, :])
            std = vt((128, 1), f"{name}_std")
            V.tensor_scalar(out=std[:, :], in0=vs[:, :], scalar1=1.0 / dim,
                            scalar2=1e-5, op0=Alu.mult, op1=Alu.add)
            S.activation(out=std[:, :], in_=std[:, :], func=Act.Sqrt)
            rstd = vt((128, 1), f"{name}_rstd")
            V.reciprocal(out=rstd[:, :], in_=std[:, :])
            o = vt((128, dim), f"{name}_o")
            V.scalar_tensor_tensor(out=o[:, :], in0=xm[:, :], scalar=rstd[:, :],
                                   in1=g_s[:, :], op0=Alu.mult, op1=Alu.mult)
            V.tensor_tensor(out=o[:, :], in0=o[:, :], in1=b_s[:, :], op=Alu.add)
            return o

        # ---------------- D: positional branch (overlaps gathers) ---------
        pos1_p = pp.tile([128, 256], f32, name="pos1_p", tag="ps")
        T.matmul(pos1_p[:, :], lhsT=rph_s[0:3, :], rhs=pw1_s[:, :], start=True, stop=True)
        p1 = vt((128, 256), "p1")
        V.scalar_tensor_tensor(out=p1[:, :], in0=pos1_p[:, :], scalar=0.0,
                               in1=pb1_s[:, :], op0=Alu.add, op1=Alu.add)
        l1 = layer_norm(p1, pg1_s, pbe1_s, 256, "ln1")
        r1a = vt((128, 256), "r1a")
        S.activation(out=r1a[:, :], in_=l1[:, :], func=Act.Relu)
        rT0, rT1 = transpose2(r1a, "rT")
        pos2_p = pp.tile([128, 256], f32, name="pos2_p", tag="ps")
        T.matmul(pos2_p[:, :], lhsT=rT0[:, :], rhs=pw2_0[:, :], start=True, stop=False)
        T.matmul(pos2_p[:, :], lhsT=rT1[:, :], rhs=pw2_1[:, :], start=False, stop=True)
        p2 = vt((128, 256), "p2")
        V.scalar_tensor_tensor(out=p2[:, :], in0=pos2_p[:, :], scalar=0.0,
                               in1=pb2_s[:, :], op0=Alu.add, op1=Alu.add)
        l2 = layer_norm(p2, pg2_s, pbe2_s, 256, "ln2")
        pos = vt((128, 256), "pos")
        S.activation(out=pos[:, :], in_=l2[:, :], func=Act.Relu)

        # ---------------- E: per-camera scale + reduce on the PE ----------
        # psum_out += diag(w_all[:, 16c+rh]) @ G_rh for each (cam, slot):
        # applies per-(query,slot) weights and sums slots/cameras in PSUM.
        # float32r single-pass matmuls keep PE at 1 cycle/row; operands are
        # declared float32r so the BIR verifier sees rounded producers.
        f32r = mybir.dt.float32r
        psum_out = pp.tile([128, 256], f32, name="psum_out", tag="psout", bufs=1)
        for cam in range(N):
            g_t = g_tiles[cam]
            diag = gp.tile([128, 2048], f32r, name=f"diag{cam}", tag="diag", bufs=2)
            V.scalar_tensor_tensor(
                out=_sub(diag, 0, [[128, 16], [1, 128]]),
                in0=_sub(i16x_s, 0, [[128, 16], [1, 128]]),
                scalar=0.0,
                in1=_sub(w_all, 16 * cam, [[1, 16], [0, 128]]),
                op0=Alu.add, op1=Alu.mult)
            for rh in range(16):
                T.matmul(psum_out[:, :],
                         lhsT=diag[:, 128 * rh:128 * rh + 128],
                         rhs=g_t[:, 256 * rh:256 * rh + 256],
                         start=(cam == 0 and rh == 0),
                         stop=(cam == N - 1 and rh == 15))
        out_acc = vt((128, 256), "out_acc")
        V.tensor_copy(out=out_acc[:, :], in_=psum_out[:, :])
        oT0, oT1 = transpose2(out_acc, "oT")
        outw_p = pp.tile([128, 256], f32, name="outw_p", tag="ps")
        T.matmul(outw_p[:, :], lhsT=oT0[:, :], rhs=wout0[:, :], start=True, stop=False)
        T.matmul(outw_p[:, :], lhsT=oT1[:, :], rhs=wout1[:, :], start=False, stop=True)

        # ---------------- F: tail -----------------------------------------
        ssum = vt((128, 256), "ssum")
        V.scalar_tensor_tensor(out=ssum[:, :], in0=outw_p[:, :], scalar=0.0,
                               in1=bout_s[:, :], op0=Alu.add, op1=Alu.add)
        V.tensor_tensor(out=ssum[:, :], in0=ssum[:, :], in1=qe[:, :], op=Alu.add)
        V.tensor_tensor(out=ssum[:, :], in0=ssum[:, :], in1=pos[:, :], op=Alu.add)
        sT0, sT1 = transpose2(ssum, "sT")
        fin_p = pp.tile([128, 64], f32, name="fin_p", tag="ps")
        T.matmul(fin_p[:, :], lhsT=sT0[:, :], rhs=wfin0[:, :], start=True, stop=False)
        T.matmul(fin_p[:, :], lhsT=sT1[:, :], rhs=wfin1[:, :], start=False, stop=True)
        f1 = vt((128, 64), "f1")
        V.scalar_tensor_tensor(out=f1[:, :], in0=fin_p[:, :], scalar=0.0,
                               in1=bfin_s[:, :], op0=Alu.add, op1=Alu.add)
        fo = layer_norm(f1, gn_s, bn_s, 64, "ln3")
        nc.sync.dma_start(out=out_d, in_=fo[:, :])

    nc.compile()
    return nc


# ---------------------------------------------------------------- entry
def _ensure_ntff_hook():
    """Register the axon NTFF profiling hook if the image lacks antenv.axon_hooks."""
    import sys
    import types
    try:
        import antenv.axon_hooks  # noqa: F401
        return
    except ImportError:
        pass
    m = types.ModuleType("antenv.axon_hooks")
    _h = [None]
    m.set_axon_ntff_profile_hook = lambda h: _h.__setitem__(0, h)
    m.get_axon_ntff_profile_hook = lambda: _h[0]
    sys.modules["antenv.axon_hooks"] = m
    try:
        import antenv
        antenv.axon_hooks = m
    except ImportError:
        pass
    try:
        from trn_agent_boot.trn_boot import _ntff_profile_via_ctypes
        hook = _ntff_profile_via_ctypes("/opt/axon/libaxon_pjrt.so")
        if hook is not None:
            m.set_axon_ntff_profile_hook(hook)
    except Exception:
        pass


def kernel(**inputs):
    if "nc" not in _CACHE:
        _CACHE["nc"] = build_nc()
    nc = _CACHE["nc"]
    in_maps = make_in_maps(inputs)
    if _CACHE.get("trace"):
        _ensure_ntff_hook()
    from concourse.bass_utils import run_bass_kernel_spmd
    res = run_bass_kernel_spmd(nc, in_maps, core_ids=list(range(NCORES)),
                               trace=bool(_CACHE.get("trace")),
                               tmpdir=_CACHE.get("tmpdir"))
    _CACHE["last_results"] = res
    out = np.concatenate([res.results[ci]["out"] for ci in range(NCORES)], axis=0)
    return out.reshape(Q, B, 64).astype(np.float32)
